# revision 1
# baseline (speedup 1.0000x reference)
"""Self-contained Trainium2 Bass kernel for nn_EpsilonModel_16973710753852.

kernel(**inputs) takes the FULL unsharded inputs (as produced by
setup_inputs()), shards the batch (B=32) across 8 NeuronCores (4 samples
each), runs a Bass/Tile kernel per core (full 3-layer Mamba model incl.
selective scan via the DVE tensor_tensor_scan instruction), and gathers
the full [32, 2] output.
"""
import sys
sys.path.insert(0, "/opt/trn_rl_repo")

import numpy as np
from contextlib import ExitStack

import jax
from jax.sharding import Mesh, PartitionSpec
from jax.experimental.shard_map import shard_map

import concourse.bass as bass
import concourse.tile as tile
from concourse import bacc, mybir
from concourse.bass2jax import (_bass_exec_p, install_neuronx_cc_hook,
                                partition_id_tensor)

F32 = mybir.dt.float32
F32R = mybir.dt.float32r
BF16 = mybir.dt.bfloat16
AF = mybir.ActivationFunctionType
OP = mybir.AluOpType

D_MODEL = 128
D_INNER = 512
D_STATE = 16
D_CONV = 8
DT_RANK = 8


def r32(ap):
    return ap.bitcast(F32R)


def build(B_local=4, S=1024, n_layers=3, debug_dump=(), CH=512,
          pool_bmul=True):
    CH = min(CH, S)
    TB = B_local * S
    NCH = S // CH
    DT_TILES = D_INNER // 128
    nc = bacc.Bacc("TRN2", target_bir_lowering=False, debug=False)

    def din(name, shape):
        return nc.dram_tensor(name, shape, F32, kind="ExternalInput").ap()

    xnT = din("xnT", [4, TB])
    fcT = din("fcT", [4, D_MODEL])
    fcb = din("fcb", [D_MODEL, 1])
    L = []
    for i in range(n_layers):
        L.append(dict(
            linT=din(f"linT{i}", [D_MODEL, D_MODEL]),
            linb=din(f"linb{i}", [D_MODEL, 1]),
            inpT=din(f"inpT{i}", [D_MODEL, 2 * D_INNER]),
            convw=din(f"convw{i}", [D_INNER, D_CONV]),
            convb=din(f"convb{i}", [D_INNER, 1]),
            xprojdT=din(f"xprojdT{i}", [D_INNER, DT_RANK]),
            xprojBT=din(f"xprojBT{i}", [D_INNER, 128]),
            xprojCT=din(f"xprojCT{i}", [D_INNER, 128]),
            dtprojT=din(f"dtprojT{i}", [DT_RANK, D_INNER]),
            dtprojb=din(f"dtprojb{i}", [D_INNER, 1]),
            Acols=din(f"Acols{i}", [128, 64]),
            Dcol=din(f"Dcol{i}", [D_INNER, 1]),
            outprojT=din(f"outprojT{i}", [D_INNER, D_MODEL]),
        ))
    R_rep = din("R_rep", [16 * 128, 128])
    R_redF = din("R_redF", [16 * 128, 128])
    zeros7 = din("zeros7", [128, D_CONV - 1])
    I128 = din("I128", [128, 128])
    w1T = din("w1T", [D_MODEL, 512]); b1 = din("b1", [512, 1])
    w2T = din("w2T", [512, 512]); b2 = din("b2", [512, 1])
    w3T = din("w3T", [512, 2]); b3 = din("b3", [2, 1])

    out_head = nc.dram_tensor("out_head", [2, B_local], F32,
                              kind="ExternalOutput").ap()
    dbg = {}
    for name in debug_dump:
        shape = {
            "g": [128, S], "xi": [D_INNER, S], "sz": [D_INNER, S],
            "dt": [D_INNER, S], "u": [D_INNER, S],
            "dA0": [128, S], "b0": [128, S], "hsc0": [128, S],
            "y": [D_INNER, S], "yg": [D_INNER, S], "hout": [128, S],
        }[name]
        dbg[name] = nc.dram_tensor(f"dbg_{name}", shape, F32,
                                   kind="ExternalOutput").ap()

    with tile.TileContext(nc) as tc, ExitStack() as ctx:
        wp = ctx.enter_context(tc.tile_pool(name="weights", bufs=1))
        cp = ctx.enter_context(tc.tile_pool(name="consts", bufs=1))
        ap_ = ctx.enter_context(tc.tile_pool(name="acts", bufs=1))
        sp = ctx.enter_context(tc.tile_pool(name="scan", bufs=2))
        tp = ctx.enter_context(tc.tile_pool(name="tmp", bufs=2))
        psA = ctx.enter_context(tc.tile_pool(name="psA", bufs=2, space="PSUM"))
        psB = ctx.enter_context(tc.tile_pool(name="psB", bufs=1, space="PSUM"))
        psY = ctx.enter_context(tc.tile_pool(name="psY", bufs=1, space="PSUM"))

        # ---- persistent consts ----
        t_Rjj = []
        for jj in range(16):
            t = cp.tile([128, 128], F32, name=f"Rjj{jj}", tag=f"Rjj{jj}")
            nc.gpsimd.dma_start(r32(t[:]), R_rep[jj * 128:(jj + 1) * 128, :])
            t_Rjj.append(t)
        t_RredF = []
        for v in range(16):
            t = cp.tile([128, 128], BF16, name=f"Rred{v}", tag=f"Rred{v}")
            nc.gpsimd.dma_start(t[:], R_redF[v * 128:(v + 1) * 128, :])
            t_RredF.append(t)
        t_I = cp.tile([128, 128], F32, name="I128", tag="I128")
        nc.sync.dma_start(t_I[:], I128)
        t_fcT = cp.tile([4, D_MODEL], F32, name="fcT", tag="fcT")
        nc.gpsimd.dma_start(r32(t_fcT[:]), fcT)
        t_fcb = cp.tile([D_MODEL, 1], F32, name="fcb", tag="fcb")
        nc.sync.dma_start(t_fcb[:], fcb)
        h_full = cp.tile([128, TB], F32, name="h_full", tag="h_full")

        # ---- embed ----
        for c0 in range(0, TB, CH):
            t_xc = tp.tile([4, CH], F32, name="xnc", tag="xnc")
            nc.gpsimd.dma_start(r32(t_xc[:]), xnT[:, c0:c0 + CH])
            ps = psA.tile([128, CH], F32, name="psA", tag="psA")
            nc.tensor.matmul(ps[:], r32(t_fcT[:]), r32(t_xc[:]),
                             start=True, stop=True)
            nc.scalar.activation(r32(h_full[:, c0:c0 + CH]), ps[:],
                                 AF.Identity, bias=t_fcb[:])

        for li in range(n_layers):
            W = L[li]
            t_linT = wp.tile([128, 128], F32, name="linT", tag="linT")
            nc.gpsimd.dma_start(r32(t_linT[:]), W["linT"])
            t_linb = wp.tile([128, 1], F32, name="linb", tag="linb")
            nc.sync.dma_start(t_linb[:], W["linb"])
            t_inpT = wp.tile([128, 2 * D_INNER], F32, name="inpT", tag="inpT")
            nc.gpsimd.dma_start(r32(t_inpT[:]), W["inpT"])
            t_convw, t_convb, t_xpT, t_dtb, t_Dcol, t_opT = [], [], [], [], [], []
            t_xpBT, t_xpCT = [], []
            for j in range(DT_TILES):
                t = wp.tile([128, D_CONV], F32, name=f"convw{j}", tag=f"convw{j}")
                nc.sync.dma_start(t[:], W["convw"][j * 128:(j + 1) * 128, :])
                t_convw.append(t)
                t = wp.tile([128, 1], F32, name=f"convb{j}", tag=f"convb{j}")
                nc.sync.dma_start(t[:], W["convb"][j * 128:(j + 1) * 128, :])
                t_convb.append(t)
                t = wp.tile([128, DT_RANK], F32, name=f"xpT{j}", tag=f"xpT{j}")
                nc.gpsimd.dma_start(r32(t[:]), W["xprojdT"][j * 128:(j + 1) * 128, :])
                t_xpT.append(t)
                t = wp.tile([128, 128], F32, name=f"xpBT{j}", tag=f"xpBT{j}")
                nc.gpsimd.dma_start(r32(t[:]), W["xprojBT"][j * 128:(j + 1) * 128, :])
                t_xpBT.append(t)
                t = wp.tile([128, 128], F32, name=f"xpCT{j}", tag=f"xpCT{j}")
                nc.gpsimd.dma_start(r32(t[:]), W["xprojCT"][j * 128:(j + 1) * 128, :])
                t_xpCT.append(t)
                t = wp.tile([128, 1], F32, name=f"dtb{j}", tag=f"dtb{j}")
                nc.sync.dma_start(t[:], W["dtprojb"][j * 128:(j + 1) * 128, :])
                t_dtb.append(t)
                t = wp.tile([128, 1], F32, name=f"Dcol{j}", tag=f"Dcol{j}")
                nc.sync.dma_start(t[:], W["Dcol"][j * 128:(j + 1) * 128, :])
                t_Dcol.append(t)
                t = wp.tile([128, 128], F32, name=f"opT{j}", tag=f"opT{j}")
                nc.gpsimd.dma_start(r32(t[:]), W["outprojT"][j * 128:(j + 1) * 128, :])
                t_opT.append(t)
            t_dtpT = wp.tile([DT_RANK, D_INNER], F32, name="dtpT", tag="dtpT")
            nc.gpsimd.dma_start(r32(t_dtpT[:]), W["dtprojT"])
            t_Acols = wp.tile([128, 64], F32, name="Acols", tag="Acols")
            nc.sync.dma_start(t_Acols[:], W["Acols"])

            t_diag = []
            for j in range(DT_TILES):
                row = []
                for k in range(D_CONV):
                    d = tp.tile([128, 128], F32, name=f"diag{j}_{k}",
                                tag=f"diag{j}_{k}", bufs=1)
                    nc.vector.tensor_scalar_mul(r32(d[:]), t_I[:],
                                                t_convw[j][:, k:k + 1])
                    row.append(d)
                t_diag.append(row)

            for s in range(B_local):
                tok0 = s * S
                dump = dbg if (li == 0 and s == 0) else {}

                # -- g = tanh(lin h + b) --
                t_g = ap_.tile([128, S], F32, name="g", tag="g")
                for c0 in range(0, S, CH):
                    ps = psA.tile([128, CH], F32, name="psA", tag="psA")
                    nc.tensor.matmul(ps[:], r32(t_linT[:]),
                                     r32(h_full[:, tok0 + c0:tok0 + c0 + CH]),
                                     start=True, stop=True)
                    nc.scalar.activation(r32(t_g[:, c0:c0 + CH]), ps[:],
                                         AF.Tanh, bias=t_linb[:])
                if "g" in dump:
                    nc.sync.dma_start(dump["g"], t_g[:])

                # -- in_proj: xi_raw (padded) + sz = silu(z) --
                t_xir = [ap_.tile([128, D_CONV - 1 + S], F32,
                                  name=f"xir{j}", tag=f"xir{j}")
                         for j in range(DT_TILES)]
                t_sz = [ap_.tile([128, S], F32, name=f"sz{j}", tag=f"sz{j}")
                        for j in range(DT_TILES)]
                for j in range(DT_TILES):
                    nc.gpsimd.dma_start(r32(t_xir[j][:, 0:D_CONV - 1]), zeros7)
                for mt in range(8):
                    for c0 in range(0, S, CH):
                        ps = psA.tile([128, CH], F32, name="psA", tag="psA")
                        nc.tensor.matmul(
                            ps[:], r32(t_inpT[:, mt * 128:(mt + 1) * 128]),
                            r32(t_g[:, c0:c0 + CH]), start=True, stop=True)
                        if mt < 4:
                            nc.scalar.activation(
                                r32(t_xir[mt][:, D_CONV - 1 + c0:
                                              D_CONV - 1 + c0 + CH]),
                                ps[:], AF.Identity)
                        else:
                            nc.scalar.activation(t_sz[mt - 4][:, c0:c0 + CH],
                                                 ps[:], AF.Silu)

                # -- conv + silu -> xi --
                t_xi = [ap_.tile([128, S], F32, name=f"xi{j}", tag=f"xi{j}")
                        for j in range(DT_TILES)]
                for j in range(DT_TILES):
                    for c0 in range(0, S, CH):
                        psc = psA.tile([128, CH], F32, name="psConv",
                                       tag="psA")
                        for k in range(D_CONV):
                            nc.tensor.matmul(
                                psc[:], r32(t_diag[j][k][:]),
                                r32(t_xir[j][:, c0 + k:c0 + k + CH]),
                                start=(k == 0), stop=(k == D_CONV - 1))
                        nc.scalar.activation(r32(t_xi[j][:, c0:c0 + CH]),
                                             psc[:], AF.Silu,
                                             bias=t_convb[j][:])
                if "xi" in dump:
                    for j in range(DT_TILES):
                        nc.sync.dma_start(dump["xi"][j * 128:(j + 1) * 128, :],
                                          t_xi[j][:])
                if "sz" in dump:
                    for j in range(DT_TILES):
                        nc.sync.dma_start(dump["sz"][j * 128:(j + 1) * 128, :],
                                          t_sz[j][:])

                # -- dtr8 = x_proj[:8] @ xi (f32r) --
                t_dtr8 = ap_.tile([8, S], F32, name="dtr8", tag="dtr8")
                for c0 in range(0, S, CH):
                    ps = psA.tile([8, CH], F32, name="psDbl", tag="psA")
                    for kt in range(DT_TILES):
                        nc.tensor.matmul(ps[:], r32(t_xpT[kt][:]),
                                         r32(t_xi[kt][:, c0:c0 + CH]),
                                         start=(kt == 0), stop=(kt == 3))
                    nc.scalar.activation(r32(t_dtr8[:, c0:c0 + CH]), ps[:],
                                         AF.Identity)

                # -- dt = softplus(dt_proj @ dtr8 + b) (f32r) --
                t_dt = [ap_.tile([128, S], F32, name=f"dt{j}", tag=f"dt{j}")
                        for j in range(DT_TILES)]
                for j in range(DT_TILES):
                    for c0 in range(0, S, CH):
                        ps = psA.tile([128, CH], F32, name="psA", tag="psA")
                        nc.tensor.matmul(ps[:],
                                         r32(t_dtpT[:, j * 128:(j + 1) * 128]),
                                         r32(t_dtr8[:, c0:c0 + CH]),
                                         start=True, stop=True)
                        t_e = tp.tile([128, CH], F32, name="dte", tag="dte",
                                      bufs=1)
                        nc.scalar.activation(t_e[:], ps[:], AF.Exp,
                                             bias=t_dtb[j][:])
                        nc.scalar.activation(r32(t_dt[j][:, c0:c0 + CH]),
                                             t_e[:], AF.Ln, bias=1.0)
                if "dt" in dump:
                    for j in range(DT_TILES):
                        nc.sync.dma_start(dump["dt"][j * 128:(j + 1) * 128, :],
                                          t_dt[j][:])

                # -- u = dt*xi --
                t_u = [ap_.tile([128, S], F32, name=f"u{j}", tag=f"u{j}")
                       for j in range(DT_TILES)]
                for j in range(DT_TILES):
                    nc.vector.tensor_mul(r32(t_u[j][:]), t_dt[j][:], t_xi[j][:])
                if "u" in dump:
                    for j in range(DT_TILES):
                        nc.sync.dma_start(dump["u"][j * 128:(j + 1) * 128, :],
                                          t_u[j][:])

                # -- B_rep / C_rep (bf16) --
                t_Brep = ap_.tile([128, S], BF16, name="Brep", tag="Brep")
                t_Crep = ap_.tile([128, S], BF16, name="Crep", tag="Crep")
                for c0 in range(0, S, CH):
                    ps = psA.tile([128, CH], F32, name="psA", tag="psA")
                    for kt in range(DT_TILES):
                        nc.tensor.matmul(ps[:], r32(t_xpBT[kt][:]),
                                         r32(t_xi[kt][:, c0:c0 + CH]),
                                         start=(kt == 0), stop=(kt == 3))
                    nc.scalar.copy(t_Brep[:, c0:c0 + CH], ps[:])
                    ps2 = psA.tile([128, CH], F32, name="psA", tag="psA")
                    for kt in range(DT_TILES):
                        nc.tensor.matmul(ps2[:], r32(t_xpCT[kt][:]),
                                         r32(t_xi[kt][:, c0:c0 + CH]),
                                         start=(kt == 0), stop=(kt == 3))
                    nc.scalar.copy(t_Crep[:, c0:c0 + CH], ps2[:])

                # -- scan lanes --
                t_yg = [ap_.tile([128, S], F32, name=f"yg{j}", tag=f"dt{j}")
                        for j in range(DT_TILES)]
                for j in range(DT_TILES):
                    yp = psY.tile([128, S], F32, name="psYa", tag="psYa",
                                  bufs=1)
                    for jj in range(16):
                        lt = j * 16 + jj
                        t_dA = sp.tile([128, S], F32, name="dA", tag="dA")
                        t_b = sp.tile([128, S], BF16, name="b", tag="b")
                        for c0 in range(0, S, CH):
                            psdt = psB.tile([128, CH], F32, name="psDt",
                                            tag="psDt", bufs=2)
                            nc.tensor.matmul(psdt[:],
                                             r32(t_Rjj[jj][:]),
                                             r32(t_dt[j][:, c0:c0 + CH]),
                                             start=True, stop=True)
                            nc.scalar.activation(t_dA[:, c0:c0 + CH], psdt[:],
                                                 AF.Exp,
                                                 scale=t_Acols[:, lt:lt + 1])
                            psu = psB.tile([128, CH], F32, name="psU",
                                           tag="psU", bufs=2)
                            nc.tensor.matmul(psu[:],
                                             r32(t_Rjj[jj][:]),
                                             r32(t_u[j][:, c0:c0 + CH]),
                                             start=True, stop=True)
                            nc.vector.tensor_mul(t_b[:, c0:c0 + CH], psu[:],
                                                 t_Brep[:, c0:c0 + CH])
                        t_h = sp.tile([128, S], BF16, name="hsc", tag="hsc")
                        nc.vector.tensor_tensor_scan(
                            t_h[:], t_dA[:], t_b[:], 0.0, OP.mult, OP.add)
                        t_ym = sp.tile([128, S], BF16, name="ym", tag="ym",
                                       bufs=3)
                        if pool_bmul:
                            nc.gpsimd.tensor_mul(t_ym[:], t_h[:], t_Crep[:])
                        else:
                            nc.vector.tensor_mul(t_ym[:], t_h[:], t_Crep[:])
                        if lt == 0 and "dA0" in dump:
                            nc.sync.dma_start(dump["dA0"], t_dA[:])
                        if lt == 0 and "b0" in dump:
                            nc.gpsimd.dma_start(dump["b0"], t_b[:])
                        if lt == 0 and "hsc0" in dump:
                            nc.gpsimd.dma_start(dump["hsc0"], t_h[:])
                        for c0 in range(0, S, CH):
                            nc.tensor.matmul(
                                yp[:, c0:c0 + CH], t_RredF[jj][:],
                                t_ym[:, c0:c0 + CH],
                                start=(jj == 0), stop=(jj == 15))
                    # gating
                    for c0 in range(0, S, CH):
                        t_q = tp.tile([128, CH], F32, name="q", tag="q")
                        nc.vector.scalar_tensor_tensor(
                            t_q[:], t_xi[j][:, c0:c0 + CH], t_Dcol[j][:],
                            yp[:, c0:c0 + CH], OP.mult, OP.add)
                        nc.vector.tensor_mul(r32(t_yg[j][:, c0:c0 + CH]),
                                             t_q[:], t_sz[j][:, c0:c0 + CH])
                        if "y" in dump:
                            t_ycp = tp.tile([128, CH], F32, name="ycp",
                                            tag="ycp", bufs=1)
                            nc.scalar.copy(t_ycp[:], yp[:, c0:c0 + CH])
                            nc.sync.dma_start(
                                dump["y"][j * 128:(j + 1) * 128, c0:c0 + CH],
                                t_ycp[:])
                if "yg" in dump:
                    for j in range(DT_TILES):
                        nc.sync.dma_start(
                            dump["yg"][j * 128:(j + 1) * 128, :], t_yg[j][:])

                # -- h = relu(out_proj @ yg) --
                for c0 in range(0, S, CH):
                    ps = psA.tile([128, CH], F32, name="psA", tag="psA")
                    for kt in range(DT_TILES):
                        nc.tensor.matmul(ps[:], r32(t_opT[kt][:]),
                                         r32(t_yg[kt][:, c0:c0 + CH]),
                                         start=(kt == 0), stop=(kt == 3))
                    nc.scalar.activation(r32(h_full[:, tok0 + c0:tok0 + c0 + CH]),
                                         ps[:], AF.Relu)
                if "hout" in dump:
                    t_hc = tp.tile([128, S], F32, name="hcp", tag="hcp",
                                   bufs=1)
                    nc.vector.tensor_copy(t_hc[:], h_full[:, tok0:tok0 + S])
                    nc.sync.dma_start(dump["hout"], t_hc[:])

        # ---- head ----
        t_w1T = cp.tile([D_MODEL, 512], F32, name="w1T", tag="g")
        nc.sync.dma_start(t_w1T[:], w1T)
        t_w2T = []
        for kt in range(4):
            t = cp.tile([128, 512], F32, name=f"w2T{kt}", tag=f"sz{kt}")
            nc.sync.dma_start(t[:], w2T[kt * 128:(kt + 1) * 128, :])
            t_w2T.append(t)
        t_w3T = []
        for kt in range(4):
            t = cp.tile([128, 2], F32, name=f"w3T{kt}", tag=f"w3T{kt}")
            nc.sync.dma_start(t[:], w3T[kt * 128:(kt + 1) * 128, :])
            t_w3T.append(t)
        t_b1, t_b2 = [], []
        for j in range(4):
            t = cp.tile([128, 1], F32, name=f"b1_{j}", tag=f"b1_{j}")
            nc.sync.dma_start(t[:], b1[j * 128:(j + 1) * 128, :])
            t_b1.append(t)
            t = cp.tile([128, 1], F32, name=f"b2_{j}", tag=f"b2_{j}")
            nc.sync.dma_start(t[:], b2[j * 128:(j + 1) * 128, :])
            t_b2.append(t)
        t_b3 = cp.tile([2, 1], F32, name="b3", tag="b3")
        nc.sync.dma_start(t_b3[:], b3)

        t_t3 = cp.tile([128, B_local], F32, name="t3", tag="t3")
        for s in range(B_local):
            nc.vector.tensor_copy(t_t3[:, s:s + 1],
                                  h_full[:, s * S + S - 1:s * S + S])

        def lrelu(ps_ap, bias_t, out_t):
            tv = tp.tile(out_t.shape, F32, name="hv", tag="hv")
            nc.scalar.activation(tv[:], ps_ap, AF.Identity, bias=bias_t[:])
            tv2 = tp.tile(out_t.shape, F32, name="hv2", tag="hv2")
            nc.vector.tensor_scalar_mul(tv2[:], tv[:], 0.01)
            nc.vector.tensor_max(out_t[:], tv[:], tv2[:])

        t_h1 = [cp.tile([128, B_local], F32, name=f"h1_{m}", tag=f"h1_{m}")
                for m in range(4)]
        for m in range(4):
            ps = psA.tile([128, B_local], F32, name="psHead", tag="psA")
            nc.tensor.matmul(ps[:], t_w1T[:, m * 128:(m + 1) * 128], t_t3[:],
                             start=True, stop=True)
            lrelu(ps[:], t_b1[m], t_h1[m])
        t_h2 = [cp.tile([128, B_local], F32, name=f"h2_{m}", tag=f"h2_{m}")
                for m in range(4)]
        for m in range(4):
            ps = psA.tile([128, B_local], F32, name="psHead", tag="psA")
            for kt in range(4):
                nc.tensor.matmul(ps[:], t_w2T[kt][:, m * 128:(m + 1) * 128],
                                 t_h1[kt][:], start=(kt == 0), stop=(kt == 3))
            lrelu(ps[:], t_b2[m], t_h2[m])
        ps = psA.tile([2, B_local], F32, name="psOut", tag="psA")
        for kt in range(4):
            nc.tensor.matmul(ps[:], t_w3T[kt][:], t_h2[kt][:],
                             start=(kt == 0), stop=(kt == 3))
        t_out = cp.tile([2, B_local], F32, name="outsb", tag="outsb")
        nc.scalar.activation(t_out[:], ps[:], AF.Identity, bias=t_b3[:])
        nc.sync.dma_start(out_head, t_out[:])

    nc.compile()
    return nc


def host_inputs(inputs, core_id, n_cores=8, B_local=4, S=1024, n_layers=3):
    f = np.float32
    x = inputs["x"].astype(f)
    start_max = x[:, :, 2].max()
    xn = np.stack([x[:, :, 0] / 255.0, x[:, :, 1] / 255.0,
                   x[:, :, 2] / start_max, x[:, :, 3]], axis=-1).astype(f)
    xs = xn[core_id * B_local:(core_id + 1) * B_local, :S]
    xnT = xs.reshape(B_local * S, 4).T.copy()

    m = {"xnT": xnT,
         "fcT": inputs["fc_w"].T.astype(f).copy(),
         "fcb": inputs["fc_b"].astype(f).reshape(-1, 1)}
    for i in range(n_layers):
        A = -np.exp(inputs["A_log"][i]).astype(f)
        Acols = np.zeros((128, 64), f)
        for lt in range(64):
            d0 = lt * 8
            Acols[:, lt] = A[d0:d0 + 8, :].reshape(128)
        m.update({
            f"linT{i}": inputs["lin_w"][i].T.astype(f).copy(),
            f"linb{i}": inputs["lin_b"][i].astype(f).reshape(-1, 1),
            f"inpT{i}": inputs["in_proj_w"][i].T.astype(f).copy(),
            f"convw{i}": inputs["conv_w"][i].astype(f).copy(),
            f"convb{i}": inputs["conv_b"][i].astype(f).reshape(-1, 1),
            f"xprojdT{i}": inputs["x_proj_w"][i].T[:, :8].astype(f).copy(),
            f"xprojBT{i}": np.ascontiguousarray(
                inputs["x_proj_w"][i].T[:, 8 + np.arange(128) % 16]).astype(f),
            f"xprojCT{i}": np.ascontiguousarray(
                inputs["x_proj_w"][i].T[:, 24 + np.arange(128) % 16]).astype(f),
            f"dtprojT{i}": inputs["dt_proj_w"][i].T.astype(f).copy(),
            f"dtprojb{i}": inputs["dt_proj_b"][i].astype(f).reshape(-1, 1),
            f"Acols{i}": Acols,
            f"Dcol{i}": inputs["D"][i].astype(f).reshape(-1, 1),
            f"outprojT{i}": inputs["out_proj_w"][i].T.astype(f).copy(),
        })
    R_rep = np.zeros((16 * 128, 128), f)
    for jj in range(16):
        for p in range(128):
            R_rep[jj * 128 + 8 * jj + p // 16, p] = 1.0
    R_redF = np.zeros((16 * 128, 128), f)
    for jj in range(16):
        for k in range(128):
            R_redF[jj * 128 + k, 8 * jj + k // 16] = 1.0
    m.update({"R_rep": R_rep, "R_redF": R_redF,
              "zeros7": np.zeros((128, 7), f),
              "I128": np.eye(128, dtype=f),
              "w1T": inputs["w1"].T.astype(f).copy(),
              "b1": inputs["b1"].astype(f).reshape(-1, 1),
              "w2T": inputs["w2"].T.astype(f).copy(),
              "b2": inputs["b2"].astype(f).reshape(-1, 1),
              "w3T": inputs["w3"].T.astype(f).copy(),
              "b3": inputs["b3"].astype(f).reshape(-1, 1)})
    return m, start_max


def make_runner(nc, n_cores=8):
    install_neuronx_cc_hook()
    in_names, out_names, out_avals, zero_outs = [], [], [], []
    partition_name = nc.partition_id_tensor.name if nc.partition_id_tensor else None
    for alloc in nc.m.functions[0].allocations:
        if not isinstance(alloc, mybir.MemoryLocationSet):
            continue
        if not alloc.memorylocations:
            continue
        name = alloc.memorylocations[0].name
        if alloc.kind == "ExternalInput":
            if name != partition_name:
                in_names.append(name)
        elif alloc.kind == "ExternalOutput":
            out_names.append(name)
            shape = tuple(alloc.tensor_shape)
            dtype = mybir.dt.np(alloc.dtype)
            out_avals.append(jax.core.ShapedArray(shape, dtype))
            zero_outs.append(np.zeros(shape, dtype))
    n_params = len(in_names)
    n_outs = len(out_avals)
    all_in_names = list(in_names) + list(out_names)
    if partition_name is not None:
        all_in_names.append(partition_name)
    donate = tuple(range(n_params, n_params + n_outs))

    def _body(*args):
        operands = list(args)
        if partition_name is not None:
            operands.append(partition_id_tensor())
        outs = _bass_exec_p.bind(
            *operands,
            out_avals=tuple(out_avals),
            in_names=tuple(all_in_names),
            out_names=tuple(out_names),
            lowering_input_output_aliases=(),
            sim_require_finite=True,
            sim_require_nnan=True,
            nc=nc,
        )
        return tuple(outs)

    devices = jax.devices()[:n_cores]
    mesh = Mesh(np.asarray(devices), ("core",))
    in_specs = (PartitionSpec("core"),) * (n_params + n_outs)
    out_specs = (PartitionSpec("core"),) * n_outs
    sharded = jax.jit(
        shard_map(_body, mesh=mesh, in_specs=in_specs, out_specs=out_specs,
                  check_rep=False),
        donate_argnums=donate, keep_unused=True)

    def run(in_maps):
        per_core = [[np.asarray(mm[name]) for name in in_names]
                    for mm in in_maps]
        concat_in = [
            np.concatenate([per_core[c][i] for c in range(n_cores)], axis=0)
            for i in range(n_params)]
        concat_zeros = [
            np.zeros((n_cores * z.shape[0], *z.shape[1:]), z.dtype)
            for z in zero_outs]
        out_arrs = sharded(*concat_in, *concat_zeros)
        out_arrs = [np.asarray(o) for o in out_arrs]
        return [
            {name: out_arrs[i].reshape(n_cores, *out_avals[i].shape)[c]
             for i, name in enumerate(out_names)}
            for c in range(n_cores)]

    def make_timed(in_maps):
        import time
        per_core = [[np.asarray(mm[name]) for name in in_names]
                    for mm in in_maps]
        concat_in = [
            np.concatenate([per_core[c][i] for c in range(n_cores)], axis=0)
            for i in range(n_params)]
        concat_zeros = [
            np.zeros((n_cores * z.shape[0], *z.shape[1:]), z.dtype)
            for z in zero_outs]
        dev_in = [jax.device_put(a) for a in concat_in]

        def timed_once():
            zz = [jax.device_put(a) for a in concat_zeros]
            for z in zz:
                z.block_until_ready()
            t0 = time.perf_counter()
            outs = sharded(*dev_in, *zz)
            for o in outs:
                o.block_until_ready()
            return time.perf_counter() - t0, outs
        return timed_once

    run.make_timed = make_timed
    return run


_CACHE = {}


def kernel(**inputs):
    n_cores, B_local = 8, 4
    if "run" not in _CACHE:
        nc = build(B_local=B_local, S=1024, n_layers=3)
        _CACHE["run"] = make_runner(nc, n_cores=n_cores)
    run = _CACHE["run"]
    in_maps = []
    start_max = None
    for c in range(n_cores):
        m, start_max = host_inputs(inputs, core_id=c, B_local=B_local)
        in_maps.append(m)
    res = run(in_maps)
    outs = [res[c]["out_head"].T for c in range(n_cores)]   # [B_local, 2] each
    out = np.concatenate(outs, axis=0).astype(np.float32)   # [32, 2]
    out = np.stack([out[:, 0] * start_max, out[:, 1]], axis=-1)
    return np.maximum(out, 0.0).astype(np.float32)

